# revision 10
# baseline (speedup 1.0000x reference)
"""Trainium2 Bass kernel for gnn_message_passing (nn_Mesh1_14267881357850).

Reference computation (N=200000, D_SPATIAL=64, D_STRUCT=131, D_OUT=256):
    out1 = concat(spatial, structural) @ W_comb.T + b_comb          [N, 256]
    agg  = (structural + structural[neighbour].sum(1)) * 0.25       [N, 131]
    out2 = agg @ W_agg.T + b_agg                                    [N, 256]
returns (out1, out2)

Strategy (8 cores, node-parallel, all-bf16, two-phase bulk gather):
  * Nodes padded to 200704 and sharded 25088/core; structural is replicated
    per-core in DRAM as sfull256 [*, 256] bf16 (rows padded 131->256 so each
    gathered row is a 512B unit, dma_gather's granularity).
  * The per-node 3-neighbour gather is NOT done with per-partition indirect
    DMAs (those cost ~1.1us of Pool-engine SWDGE time per 128 rows = 650us
    per core). Instead, per super-group of 7 node-groups (10752 fetches):
      Phase 1: the fetch indices are bucketed into NW=12 static source
        windows of WW<32k rows; one dma_gather per window (int16 indices
        relative to the window base, idx table wrapped into 16 partitions
        and replicated to all 8 Q7 cores, <=1024 idxs per instruction)
        pulls rows into a token-staging SBUF tile D1.
      Phase 2: per group, 2 SBUF-source transpose-mode dma_gathers
        (<=768 idxs each) route the group's 1536 rows from D1 into
        consumption order AND transpose them to feature-major [128,2,768]
        (feature f = s*128+p), eliminating all PE transposes.
  * VectorE sums the 3 neighbour rows feature-major and adds the self rows
    (from the feature-major a1T stream) -> aggT bf16.
  * Per 128-node tile, 4 bf16 matmuls write PSUM; ScalarE copies (f32->bf16)
    into one packed SBUF tile; one DMA per group stores to out [128,4,npc].
  * Biases ride as a ones-row in a1T (out1) and a memset ones-row in the
    agg K=4 tile (out2); 0.25 is folded into W_agg host-side.
"""

import os
import sys

import numpy as np

for _p in ("/opt/trn_rl_repo", "/root/.axon_site/_ro/trn_rl_repo"):
    if os.path.isdir(_p) and _p not in sys.path:
        sys.path.append(_p)

import ml_dtypes

import concourse.bacc as bacc
import concourse.bass as bass
import concourse.mybir as mybir
from concourse.bass_utils import run_bass_kernel_spmd
from concourse.tile import TileContext

F32 = mybir.dt.float32
BF16 = mybir.dt.bfloat16
I16 = mybir.dt.int16
NPBF = ml_dtypes.bfloat16

N = 200000
DS = 64          # spatial features
DT = 131         # structural features
ES = 256         # padded row elems in sfull256 (512B rows)
DO = 256         # output features per head
NCORES = 8
GROUP = 512      # nodes per pipeline group
SG = 7           # groups per super-group
NW = 12          # phase-1 source windows
CH2 = 768        # idxs per phase-2 instruction (2 per group)

NPC = 25088      # nodes per core (= 49 * 512)
NG = NPC // GROUP          # 49
NSG = NG // SG             # 7 super-groups
NPAD = NPC * NCORES        # 200704

KA = DS + DT + 1     # 196 rows of a1T ([spatial; structural; ones])
KB = KA - 128        # 68

# exec time of the last traced run (ns), for test harnesses
last_exec_time_ns = None


def _round_up(x, m):
    return (x + m - 1) // m * m


def wrap16_repl(vals, cap):
    """Wrap an idx list into [128, cap//16] int16: position i at
    [i%16, i//16], replicated into all 8 Q7-core partition groups."""
    arr = np.zeros((16, cap // 16), np.int16)
    v = np.asarray(vals, np.int16)
    pos = np.arange(len(v))
    arr[pos % 16, pos // 16] = v
    return np.tile(arr, (8, 1))


def build_nc(npc=NPC, n_src=N, caps=None):
    """caps: per-window phase-1 idx capacities (multiples of 128, <=1024)."""
    assert caps is not None and len(caps) == NW
    assert all(c % 128 == 0 and 0 < c <= 1024 for c in caps)
    ng = npc // GROUP
    nsg = ng // SG
    ww = _round_up(n_src + 1, NW) // NW          # window width (rows)
    assert ww <= 32768
    nsrc_pad = NW * ww
    ntok = sum(caps)                             # D1 tokens per super-group
    assert ntok <= 32767
    colbase = np.cumsum([0] + list(caps[:-1])) // 128
    n_i1_16 = ntok // 16                         # i1 cols (int16)
    n_i2_16 = (SG * 2 * CH2) // 16               # i2 cols per super-group

    nc = bacc.Bacc("TRN2", target_bir_lowering=False, debug=False)
    a1T = nc.dram_tensor("a1T", [KA, npc], BF16, kind="ExternalInput")
    sfull = nc.dram_tensor("sfull", [nsrc_pad, ES], BF16, kind="ExternalInput")
    idx1 = nc.dram_tensor("idx1", [nsg, 128, n_i1_16], I16, kind="ExternalInput")
    idx2 = nc.dram_tensor("idx2", [nsg, 128, n_i2_16], I16, kind="ExternalInput")
    w1 = nc.dram_tensor("w1", [KA, DO], BF16, kind="ExternalInput")
    w2 = nc.dram_tensor("w2", [DT + 1, DO], BF16, kind="ExternalInput")
    # packed output: out[p, cc, n]: cc 0,1 -> out1T rows (cc*128+p),
    # cc 2,3 -> out2T rows ((cc-2)*128+p)
    out = nc.dram_tensor("out", [128, 4, npc], BF16, kind="ExternalOutput")

    with TileContext(nc) as tc:
        with (
            tc.tile_pool(name="const", bufs=1) as cpool,
            tc.tile_pool(name="work", bufs=3) as wpool,
            tc.tile_pool(name="stage", bufs=2) as spool,
            tc.tile_pool(name="route", bufs=6) as rpool,
            tc.tile_pool(name="nsums", bufs=3) as npool,
            tc.tile_pool(name="osb", bufs=3) as opool,
            tc.tile_pool(name="pout", bufs=4, space="PSUM") as pout,
        ):
            # ---- constants ----
            w1a = cpool.tile([128, DO], BF16)
            nc.sync.dma_start(out=w1a, in_=w1[0:128, :])
            w1b = cpool.tile([KB, DO], BF16)
            nc.sync.dma_start(out=w1b, in_=w1[128:KA, :])
            w2a = cpool.tile([128, DO], BF16)
            nc.sync.dma_start(out=w2a, in_=w2[0:128, :])
            w2b = cpool.tile([4, DO], BF16)
            nc.sync.dma_start(out=w2b, in_=w2[128 : DT + 1, :])

            for s in range(nsg):
                # ---- phase 1: windowed bulk gather into token staging ----
                i1 = spool.tile([128, n_i1_16], I16, tag="i1")
                nc.sync.dma_start(out=i1, in_=idx1[s, :, :])
                i2 = spool.tile([128, n_i2_16], I16, tag="i2")
                nc.sync.dma_start(out=i2, in_=idx2[s, :, :])
                d1 = spool.tile([128, ntok // 128, ES], BF16, tag="d1")
                o16 = 0
                for w in range(NW):
                    cw = caps[w]
                    cb = int(colbase[w])
                    nc.gpsimd.dma_gather(
                        d1[:, cb : cb + cw // 128, :],
                        sfull[w * ww : (w + 1) * ww, :],
                        i1[:, o16 : o16 + cw // 16],
                        cw, cw, ES)
                    o16 += cw // 16

                for gg in range(SG):
                    g = s * SG + gg
                    n0 = g * GROUP

                    # ---- feature-major activation stream ----
                    a1a = wpool.tile([128, GROUP], BF16, tag="a1a")
                    nc.sync.dma_start(out=a1a, in_=a1T[0:128, n0 : n0 + GROUP])
                    a1b = wpool.tile([KB, GROUP], BF16, tag="a1b")
                    nc.sync.dma_start(out=a1b, in_=a1T[128:KA, n0 : n0 + GROUP])
                    # partition-0-aligned self structural rows (feats 0..127
                    # and 128..130) for the agg adds: DVE needs equal base
                    # partitions when both inputs live in SBUF
                    sfT = wpool.tile([128, GROUP], BF16, tag="sfT")
                    nc.sync.dma_start(
                        out=sfT, in_=a1T[DS : DS + 128, n0 : n0 + GROUP])
                    sfB = wpool.tile([3, GROUP], BF16, tag="sfB")
                    nc.sync.dma_start(
                        out=sfB, in_=a1T[DS + 128 : DS + DT, n0 : n0 + GROUP])

                    # ---- phase 2: route + transpose 1536 rows, 2 chunks ----
                    # chunk A cols: j0 n0..511, j1 n0..255
                    # chunk B cols: j1 n256..511, j2 n0..511
                    ca = rpool.tile([128, 2, CH2], BF16, tag="ca")
                    cb_t = rpool.tile([128, 2, CH2], BF16, tag="cb")
                    for k, d2c in ((0, ca), (1, cb_t)):
                        base16 = (gg * 2 + k) * (CH2 // 16)
                        nc.gpsimd.dma_gather(
                            d2c[:, :, :], d1[:, :, :],
                            i2[:, base16 : base16 + CH2 // 16],
                            CH2, CH2, ES,
                            transpose=True,
                            sbuf_tokens_per_rank=128,
                            sbuf_free_dim_per_rank=ES * 2,
                        )

                    # ---- neighbour sum (feature-major, f = s*128 + p) ----
                    ns = npool.tile([128, 2, GROUP], BF16, tag="ns")
                    nc.vector.tensor_add(
                        out=ns[:, :, 0:256], in0=ca[:, :, 0:256],
                        in1=ca[:, :, 512:768])
                    nc.vector.tensor_add(
                        out=ns[:, :, 0:256], in0=ns[:, :, 0:256],
                        in1=cb_t[:, :, 256:512])
                    nc.vector.tensor_add(
                        out=ns[:, :, 256:512], in0=ca[:, :, 256:512],
                        in1=cb_t[:, :, 0:256])
                    nc.vector.tensor_add(
                        out=ns[:, :, 256:512], in0=ns[:, :, 256:512],
                        in1=cb_t[:, :, 512:768])

                    # ---- aggT = nsumT + structT(self) ----
                    aggA = wpool.tile([128, GROUP], BF16, tag="aggA")
                    nc.vector.tensor_add(
                        out=aggA, in0=ns[:, 0, :], in1=sfT)
                    aggB = wpool.tile([4, GROUP], BF16, tag="aggB")
                    nc.vector.memset(aggB[:, :], 1.0)
                    nc.vector.tensor_add(
                        out=aggB[0:3, :], in0=ns[0:3, 1, :], in1=sfB[0:3, :])

                    # ---- matmuls + packed store ----
                    o_all = opool.tile([128, 4, GROUP], BF16, tag="oall")
                    for c in range(2):
                        csl = slice(c * 128, (c + 1) * 128)
                        p1 = pout.tile([128, GROUP], F32, tag="ps")
                        nc.tensor.matmul(
                            p1, lhsT=w1a[:, csl], rhs=a1a,
                            start=True, stop=False)
                        nc.tensor.matmul(
                            p1, lhsT=w1b[:, csl], rhs=a1b,
                            start=False, stop=True)
                        p2 = pout.tile([128, GROUP], F32, tag="ps")
                        nc.tensor.matmul(
                            p2, lhsT=w2a[:, csl], rhs=aggA,
                            start=True, stop=False)
                        nc.tensor.matmul(
                            p2, lhsT=w2b[:, csl], rhs=aggB,
                            start=False, stop=True)
                        nc.scalar.activation(
                            out=o_all[:, c, :], in_=p1,
                            func=mybir.ActivationFunctionType.Copy)
                        nc.scalar.activation(
                            out=o_all[:, 2 + c, :], in_=p2,
                            func=mybir.ActivationFunctionType.Copy)
                    nc.sync.dma_start(
                        out=out[:, :, n0 : n0 + GROUP], in_=o_all[:, :, :])
    nc.compile()
    return nc


def prep_inputs(spatial, structural, neighbour, W_agg, b_agg, W_comb, b_comb,
                npc=NPC, ncores=NCORES):
    """Host-side shard + layout transform.

    Returns (in_maps, caps): per-core input dicts and the static per-window
    phase-1 capacities the Bass program must be built with.
    """
    n = spatial.shape[0]
    npad = npc * ncores
    ng = npc // GROUP
    nsg = ng // SG
    ww = _round_up(n + 1, NW) // NW
    nsrc_pad = NW * ww

    spatial = np.asarray(spatial, dtype=np.float32)
    structural = np.asarray(structural, dtype=np.float32)
    nbr = np.asarray(neighbour, dtype=np.int64)

    pad = npad - n
    spatial_p = np.concatenate(
        [spatial, np.zeros((pad, DS), np.float32)], axis=0)
    structural_p = np.concatenate(
        [structural, np.zeros((pad, DT), np.float32)], axis=0)
    # pad-node outputs are discarded; spread their fetches uniformly so no
    # phase-1 window is skewed past its 1024-idx instruction cap
    nbr_pad = (np.arange(pad * 3, dtype=np.int64) * 104729) % n
    nbr_p = np.concatenate([nbr, nbr_pad.reshape(pad, 3)], axis=0)

    sfull = np.zeros((nsrc_pad, ES), NPBF)
    sfull[:n, :DT] = structural.astype(NPBF)

    w1 = np.concatenate(
        [np.asarray(W_comb, np.float32).T,
         np.asarray(b_comb, np.float32)[None, :]], axis=0)
    w1 = np.ascontiguousarray(w1.astype(NPBF))            # [196, 256]
    w2 = np.concatenate(
        [0.25 * np.asarray(W_agg, np.float32).T,
         np.asarray(b_agg, np.float32)[None, :]], axis=0)
    w2 = np.ascontiguousarray(w2.astype(NPBF))            # [132, 256]

    # ---- per (core, super-group): bucket fetches into windows ----
    # cons order within a super-group: i = gg*1536 + j*512 + nn
    per_sg = []           # (core, s) -> dict(window lists, tokens)
    counts_all = np.zeros((ncores, nsg, NW), np.int64)
    for c in range(ncores):
        nb = nbr_p[c * npc : (c + 1) * npc]               # [npc, 3]
        for s in range(nsg):
            blk = nb[s * SG * GROUP : (s + 1) * SG * GROUP]
            v = blk.reshape(SG, GROUP, 3).transpose(0, 2, 1).reshape(-1)
            w_of = v // ww
            rel = (v - w_of * ww).astype(np.int16)
            counts = np.bincount(w_of, minlength=NW)
            counts_all[c, s] = counts
            starts = np.zeros(NW, np.int64)
            starts[1:] = np.cumsum(counts)[:-1]
            order = np.argsort(w_of, kind="stable")
            rank = np.empty(v.size, np.int64)
            rank[order] = np.arange(v.size)
            pos = rank - starts[w_of]                      # pos within window
            per_sg.append((c, s, w_of, rel, pos, counts, order))

    caps = tuple(int(_round_up(max(1, counts_all[:, :, w].max()), 128))
                 for w in range(NW))
    assert all(cw <= 1024 for cw in caps), f"window cap blown: {caps}"
    colbase = np.cumsum([0] + list(caps[:-1])) // 128
    ntok = sum(caps)
    n_i1_16 = ntok // 16
    n_i2_16 = (SG * 2 * CH2) // 16

    idx1_all = np.zeros((ncores, nsg, 128, n_i1_16), np.int16)
    idx2_all = np.zeros((ncores, nsg, 128, n_i2_16), np.int16)
    for (c, s, w_of, rel, pos, counts, order) in per_sg:
        # phase-1 idx tables: per window, rel values in cons order, padded
        o16 = 0
        for w in range(NW):
            lst = rel[order[(w_of[order] == w)]] if False else None
            # rel values of window w in cons order:
            sel = rel[w_of == w]
            full = np.zeros(caps[w], np.int16)
            full[: len(sel)] = sel
            idx1_all[c, s, :, o16 : o16 + caps[w] // 16] = wrap16_repl(
                full, caps[w])
            o16 += caps[w] // 16
        # phase-2: token of cons position i = colbase[w]*128 + pos
        tok = (colbase[w_of] * 128 + pos).astype(np.int16)
        tok = tok.reshape(SG, 2, CH2)
        for gg in range(SG):
            for k in range(2):
                b16 = (gg * 2 + k) * (CH2 // 16)
                idx2_all[c, s, :, b16 : b16 + CH2 // 16] = wrap16_repl(
                    tok[gg, k], CH2)

    in_maps = []
    for c in range(ncores):
        sl = slice(c * npc, (c + 1) * npc)
        a1T = np.empty((KA, npc), NPBF)
        a1T[0:DS] = spatial_p[sl].T.astype(NPBF)
        a1T[DS : DS + DT] = structural_p[sl].T.astype(NPBF)
        a1T[DS + DT] = NPBF(1.0)
        in_maps.append({
            "a1T": a1T,
            "sfull": sfull,
            "idx1": idx1_all[c],
            "idx2": idx2_all[c],
            "w1": w1,
            "w2": w2,
        })
    return in_maps, caps


_NC_CACHE = {}


def kernel(spatial, structural, neighbour, W_agg, b_agg, W_comb, b_comb):
    global last_exec_time_ns
    in_maps, caps = prep_inputs(
        spatial, structural, neighbour, W_agg, b_agg, W_comb, b_comb)
    key = (NPC, N, caps)
    if key not in _NC_CACHE:
        _NC_CACHE[key] = build_nc(NPC, N, caps)
    nc = _NC_CACHE[key]

    trace = bool(int(os.environ.get("KERNEL_TRACE", "0")))
    tmpdir = os.environ.get("KERNEL_TMPDIR") or None
    res = run_bass_kernel_spmd(
        nc, in_maps, core_ids=list(range(NCORES)), trace=trace, tmpdir=tmpdir)
    last_exec_time_ns = res.exec_time_ns

    comb = np.concatenate(
        [np.asarray(r["out"], dtype=np.float32)
         .transpose(1, 0, 2).reshape(2 * DO, NPC)
         for r in res.results], axis=1)[:, :N]
    out1 = np.ascontiguousarray(comb[:DO, :].T)
    out2 = np.ascontiguousarray(comb[DO:, :].T)
    return out1, out2


# revision 21
# speedup vs baseline: 1.6442x; 1.6442x over previous
"""Trainium2 Bass kernel for gnn_message_passing (nn_Mesh1_14267881357850).

Reference computation (N=200000, D_SPATIAL=64, D_STRUCT=131, D_OUT=256):
    out1 = concat(spatial, structural) @ W_comb.T + b_comb          [N, 256]
    agg  = (structural + structural[neighbour].sum(1)) * 0.25       [N, 131]
    out2 = agg @ W_agg.T + b_agg                                    [N, 256]
returns (out1, out2)

Strategy (8 cores, node-parallel, all-bf16 dataflow):
  * Nodes padded to 200704 and sharded 25088/core; `structural` is passed
    in full (bf16) to every core as the gather source (no collectives).
  * Host pre-transposes activations to feature-major a1T = [spatialT;
    structuralT; ones] (bf16, [196, 25088]) so matmul lhsT tiles load
    straight from DRAM.
  * Neighbour rows are fetched with ONE indirect DMA per 512-node group:
    offset ap [128, 12] (3 neighbours x 4 subtiles per partition), dest
    [128, 12, 132] bf16 (132 = 131 feats + 1 overread pad elem; sfull is
    padded by one row so the overread stays in bounds). This amortizes
    the ~1.1us SWDGE fixed overhead per indirect DMA that dominated the
    12-DMAs-per-group version.
  * VectorE sums the 3 neighbour rows (node-major, bf16 2x mode), PE
    transposes the sum to feature-major PSUM (bf16: 1 cycle/row),
    VectorE adds the (already feature-major) self rows -> aggT bf16.
  * Per 128-node tile, 4 bf16 matmuls (1 cycle/row vs 4 for fp32) write
    PSUM; ScalarE copies (with f32->bf16 cast) into one packed SBUF tile
    [128, 4, 512]; ONE DMA per group stores to DRAM out [128, 4, npc].
  * Biases ride as a host-provided ones-row in a1T (out1) and a memset
    ones-row in the agg K=4 tile (out2); 0.25 is folded into W_agg
    host-side. Outputs return as bf16 and are upcast on host (rel tol
    2e-2 >> bf16 rounding).
"""

import os
import sys

import numpy as np

for _p in ("/opt/trn_rl_repo", "/root/.axon_site/_ro/trn_rl_repo"):
    if os.path.isdir(_p) and _p not in sys.path:
        sys.path.append(_p)

import ml_dtypes

import concourse.bacc as bacc
import concourse.bass as bass
import concourse.mybir as mybir
from concourse.bass_utils import run_bass_kernel_spmd
from concourse.masks import make_identity
from concourse.tile import TileContext

F32 = mybir.dt.float32
BF16 = mybir.dt.bfloat16
I32 = mybir.dt.int32
NPBF = ml_dtypes.bfloat16

N = 200000
DS = 64          # spatial features
DT = 131         # structural features
DTP = DT + 1     # gathered elems per index (1 pad elem from row overread)
DO = 256         # output features per head
NCORES = 8
GROUP = 512      # nodes per pipeline group
SUBT = GROUP // 128   # 128-node subtiles per group
NIDX = 3 * SUBT       # gather offsets per partition per group

NPC = 25088      # nodes per core (= 49 * 512)
NG = NPC // GROUP
NPAD = NPC * NCORES  # 200704

KA = DS + DT + 1     # 196 rows of a1T ([spatial; structural; ones])
KB = KA - 128        # 68

# exec time of the last traced run (ns), for test harnesses
last_exec_time_ns = None


def build_nc(npc=NPC, n_src=N, group=GROUP):
    """Build the Bass module for one core processing `npc` nodes."""
    ng = npc // group
    subt = group // 128
    nidx = 3 * subt              # indices per partition per group

    nc = bacc.Bacc("TRN2", target_bir_lowering=False, debug=False)
    a1T = nc.dram_tensor("a1T", [KA, npc], BF16, kind="ExternalInput")
    # +1 pad row: each gathered index reads DTP=132 contiguous elems
    sfull = nc.dram_tensor("sfull", [n_src + 1, DT], BF16, kind="ExternalInput")
    idx = nc.dram_tensor("idx", [128, ng * nidx], I32, kind="ExternalInput")
    w1 = nc.dram_tensor("w1", [KA, DO], BF16, kind="ExternalInput")
    w2 = nc.dram_tensor("w2", [DT + 1, DO], BF16, kind="ExternalInput")
    # packed output: out[p, cc, n]: cc 0,1 -> out1T rows (cc*128+p),
    # cc 2,3 -> out2T rows ((cc-2)*128+p)
    out = nc.dram_tensor("out", [128, 4, npc], BF16, kind="ExternalOutput")

    with TileContext(nc) as tc:
        with (
            tc.tile_pool(name="const", bufs=1) as cpool,
            tc.tile_pool(name="work", bufs=3) as wpool,
            tc.tile_pool(name="gath", bufs=3) as gpool,
            tc.tile_pool(name="nsums", bufs=10) as npool,
            tc.tile_pool(name="osb", bufs=3) as opool,
            tc.tile_pool(name="pst", bufs=2, space="PSUM") as pst,
            tc.tile_pool(name="pout", bufs=4, space="PSUM") as pout,
        ):
            # ---- constants ----
            ident = cpool.tile([128, 128], BF16)
            make_identity(nc, ident)
            w1a = cpool.tile([128, DO], BF16)
            nc.sync.dma_start(out=w1a, in_=w1[0:128, :])
            w1b = cpool.tile([KB, DO], BF16)
            nc.sync.dma_start(out=w1b, in_=w1[128:KA, :])
            w2a = cpool.tile([128, DO], BF16)
            nc.sync.dma_start(out=w2a, in_=w2[0:128, :])
            w2b = cpool.tile([4, DO], BF16)
            nc.sync.dma_start(out=w2b, in_=w2[128 : DT + 1, :])
            idx_sb = cpool.tile([128, ng * nidx], I32)
            nc.sync.dma_start(out=idx_sb, in_=idx[:, :])

            for g in range(ng):
                n0 = g * group

                # ---- loads (feature-major activations) ----
                a1a = wpool.tile([128, group], BF16, tag="a1a")
                nc.sync.dma_start(out=a1a, in_=a1T[0:128, n0 : n0 + group])
                a1b = wpool.tile([KB, group], BF16, tag="a1b")
                nc.sync.dma_start(out=a1b, in_=a1T[128:KA, n0 : n0 + group])

                # ---- indirect gathers: one DMA per (subtile, neighbour)
                # [multi-offset per partition is NOT supported by HW DGE]
                gt = gpool.tile([128, nidx, DTP], BF16, tag="gt")
                for j in range(nidx):
                    nc.gpsimd.indirect_dma_start(
                        out=gt[:, j, 0:DT],
                        out_offset=None,
                        in_=sfull[:, :],
                        in_offset=bass.IndirectOffsetOnAxis(
                            ap=idx_sb[:, g * nidx + j : g * nidx + j + 1], axis=0
                        ),
                    )

                # ---- neighbour sum on VectorE, then PE transposes ----
                psA = pst.tile([128, group], BF16, tag="psA")
                psB = pst.tile([3, group], BF16, tag="psB")
                for b in range(subt):
                    nsum = npool.tile([128, DTP], BF16, tag="nsum")
                    nc.vector.tensor_add(
                        out=nsum[:, 0:DT], in0=gt[:, 3 * b, 0:DT],
                        in1=gt[:, 3 * b + 1, 0:DT])
                    nc.vector.tensor_add(
                        out=nsum[:, 0:DT], in0=nsum[:, 0:DT],
                        in1=gt[:, 3 * b + 2, 0:DT])
                    nc.tensor.transpose(
                        psA[:, b * 128 : (b + 1) * 128],
                        nsum[:, 0:128],
                        ident,
                    )
                    nc.tensor.transpose(
                        psB[0:3, b * 128 : (b + 1) * 128],
                        nsum[:, 128:DT],
                        ident,
                    )

                # ---- aggT = nsumT + structT(self), feature-major ----
                # structural feats 0..63 live in a1a rows 64..127,
                # feats 64..127 in a1b rows 0..63, feats 128..130 in a1b rows 64..66.
                aggA = wpool.tile([128, group], BF16, tag="aggA")
                nc.vector.tensor_add(
                    out=aggA[0:64, :], in0=psA[0:64, :], in1=a1a[64:128, :]
                )
                nc.vector.tensor_add(
                    out=aggA[64:128, :], in0=psA[64:128, :], in1=a1b[0:64, :]
                )
                aggB = wpool.tile([4, group], BF16, tag="aggB")
                # rows 0..2 overwritten below; row 3 stays 1.0 (bias ones-row)
                nc.vector.memset(aggB[:, :], 1.0)
                nc.vector.tensor_add(
                    out=aggB[0:3, :], in0=psB[0:3, :], in1=a1b[64:67, :]
                )

                # ---- matmuls (weights stationary, activations moving,
                # outputs feature-major) + packed store ----
                o_all = opool.tile([128, 4, group], BF16, tag="oall")
                for c in range(2):
                    csl = slice(c * 128, (c + 1) * 128)
                    p1 = pout.tile([128, group], F32, tag="ps")
                    nc.tensor.matmul(
                        p1, lhsT=w1a[:, csl], rhs=a1a, start=True, stop=False)
                    nc.tensor.matmul(
                        p1, lhsT=w1b[:, csl], rhs=a1b, start=False, stop=True)
                    p2 = pout.tile([128, group], F32, tag="ps")
                    nc.tensor.matmul(
                        p2, lhsT=w2a[:, csl], rhs=aggA, start=True, stop=False)
                    nc.tensor.matmul(
                        p2, lhsT=w2b[:, csl], rhs=aggB, start=False, stop=True)
                    nc.scalar.activation(
                        out=o_all[:, c, :], in_=p1,
                        func=mybir.ActivationFunctionType.Copy)
                    nc.scalar.activation(
                        out=o_all[:, 2 + c, :], in_=p2,
                        func=mybir.ActivationFunctionType.Copy)
                nc.sync.dma_start(
                    out=out[:, :, n0 : n0 + group], in_=o_all[:, :, :])
    nc.compile()
    return nc


def prep_inputs(spatial, structural, neighbour, W_agg, b_agg, W_comb, b_comb,
                npc=NPC, ncores=NCORES, group=GROUP):
    """Host-side shard + layout transform. Returns list of per-core in_maps."""
    n = spatial.shape[0]
    npad = npc * ncores
    ng = npc // group
    subt = group // 128
    nidx = 3 * subt

    spatial = np.asarray(spatial, dtype=np.float32)
    structural = np.asarray(structural, dtype=np.float32)
    nbr = np.asarray(neighbour, dtype=np.int32)

    pad = npad - n
    if pad:
        spatial_p = np.concatenate(
            [spatial, np.zeros((pad, DS), np.float32)], axis=0)
        structural_p = np.concatenate(
            [structural, np.zeros((pad, DT), np.float32)], axis=0)
        nbr_p = np.concatenate([nbr, np.zeros((pad, 3), np.int32)], axis=0)
    else:
        spatial_p, structural_p, nbr_p = spatial, structural, nbr

    # gather source: bf16, one pad row for the 132-elem overread
    sfull = np.concatenate(
        [structural, np.zeros((1, DT), np.float32)], axis=0).astype(NPBF)
    sfull = np.ascontiguousarray(sfull)

    w1 = np.concatenate(
        [np.asarray(W_comb, np.float32).T,
         np.asarray(b_comb, np.float32)[None, :]], axis=0)
    w1 = np.ascontiguousarray(w1.astype(NPBF))            # [196, 256]
    w2 = np.concatenate(
        [0.25 * np.asarray(W_agg, np.float32).T,
         np.asarray(b_agg, np.float32)[None, :]], axis=0)
    w2 = np.ascontiguousarray(w2.astype(NPBF))            # [132, 256]

    in_maps = []
    for c in range(ncores):
        sl = slice(c * npc, (c + 1) * npc)
        a1T = np.empty((KA, npc), NPBF)
        a1T[0:DS] = spatial_p[sl].T.astype(NPBF)
        a1T[DS : DS + DT] = structural_p[sl].T.astype(NPBF)
        a1T[DS + DT] = NPBF(1.0)
        # idx[p, (g*subt + b)*3 + j] = nbr[c*npc + g*group + b*128 + p, j]
        ngt = npc // 128
        idx = np.ascontiguousarray(
            nbr_p[sl].reshape(ngt, 128, 3)
            .transpose(1, 0, 2).reshape(128, ngt * 3))
        in_maps.append({
            "a1T": a1T,
            "sfull": sfull,
            "idx": idx,
            "w1": w1,
            "w2": w2,
        })
    return in_maps


_NC_CACHE = {}


def kernel(spatial, structural, neighbour, W_agg, b_agg, W_comb, b_comb):
    global last_exec_time_ns
    key = (NPC, N, GROUP)
    if key not in _NC_CACHE:
        _NC_CACHE[key] = build_nc(*key)
    nc = _NC_CACHE[key]

    in_maps = prep_inputs(
        spatial, structural, neighbour, W_agg, b_agg, W_comb, b_comb)

    trace = bool(int(os.environ.get("KERNEL_TRACE", "0")))
    tmpdir = os.environ.get("KERNEL_TMPDIR") or None
    res = run_bass_kernel_spmd(
        nc, in_maps, core_ids=list(range(NCORES)), trace=trace, tmpdir=tmpdir)
    last_exec_time_ns = res.exec_time_ns

    # res["out"] per core: [128, 4, npc] bf16 -> comb rows cc*128+p
    comb = np.concatenate(
        [np.asarray(r["out"], dtype=np.float32)
         .transpose(1, 0, 2).reshape(2 * DO, NPC)
         for r in res.results], axis=1)[:, :N]
    out1 = np.ascontiguousarray(comb[:DO, :].T)
    out2 = np.ascontiguousarray(comb[DO:, :].T)
    return out1, out2
